# revision 25
# baseline (speedup 1.0000x reference)
"""Trainium2 Bass kernel for BasicAttention.

reference math (fp32):
  xf = x.reshape(b, din, hw)               # b=4, din=256, hw=4096
  Q = q_w @ xf   [b, 64, hw]
  K = k_w @ xf   [b, 64, hw]
  V = v_w @ xf   [b, 256, hw]
  S = Q^T K      [b, hw, hw]
  A = softmax(S, axis=-1)
  z = (A @ V^T)^T -> [b, 256, h, w]

Sharding: 8 cores = (batch b in 0..4) x (query half in 0..2). Each core gets
its batch's full xf with columns rotated so its 2048 queries come first
(attention is permutation-invariant over keys, so K/V built from the rotated
xf give identical outputs).

Per-core dataflow (fp32 PSUM, matmuls in float32r):
  - k2/q2 [128, *] hold K/2 and Q duplicated on both partition halves, so S
    matmuls contract over K=128: S = (K/2)^T Q + (K/2)^T Q.  K=64 fp32r
    matmuls would be silently row-grouped into a 2x-cost two-pass mode;
    K=128 streams at 1 row/cycle.  k_w is halved on the host.
  - S psums are [128, 1024] pairs (2 banks); ONE ACT exp per pair reads
    across both banks, halving ACT instruction count.
  - Z accumulates vt^T @ exp into two [128,512] psums; at ptile end they are
    evicted raw to SBUF (freeing the banks) and normalized there.
  - Softmax denominators: DVE accumulates exp pairs 0..13 on two interleaved
    chains (Pool takes 3 early pairs); pairs 14/15 are summed directly on PE
    with ones[128,128] matmuls accumulating into the same psum as the
    ones^T accf matmul, so the boundary chain after the last Z matmul is
    just 5 short matmuls -> reciprocal_approx_fast -> normalize muls.
"""

import sys
import os

sys.path.insert(0, "/opt/trn_rl_repo")

import numpy as np

B, DIN, H, W = 4, 256, 64, 64
HW = H * W            # 4096 keys
DK, DV = 64, 256
PQ = HW // 2          # 2048 queries per core
PT = 512              # query tile (psum free dim)
QC = 128              # key chunk (contraction tile)
NPT = PQ // PT        # 4
NQC = HW // QC        # 32
PAIRS = NQC // 2      # 16
POOL_PAIRS = (1, 3, 5, 7, 9)   # pair-adds done on GpSimd
PE_PAIRS = (14, 15)        # pairs summed directly on PE at ptile end
N_CORES = 8

_cache = {}


def _build():
    if "nc" in _cache:
        return _cache["nc"]

    from contextlib import ExitStack
    import concourse.tile as tile
    from concourse import bacc, mybir

    f32 = mybir.dt.float32
    f32r = mybir.dt.float32r
    bf16 = mybir.dt.bfloat16

    nc = bacc.Bacc("TRN2", target_bir_lowering=False, debug=False,
                   num_devices=N_CORES)

    xb = nc.dram_tensor("xb", [DIN, HW], f32r, kind="ExternalInput").ap()
    qwT = nc.dram_tensor("qwT", [DIN, DK], f32r, kind="ExternalInput").ap()
    kwT = nc.dram_tensor("kwT", [DIN, DK], f32r, kind="ExternalInput").ap()
    vwT = nc.dram_tensor("vwT", [DIN, DV], f32r, kind="ExternalInput").ap()
    zout = nc.dram_tensor("zout", [DV, PQ], f32, kind="ExternalOutput").ap()

    with tile.TileContext(nc) as tc, ExitStack() as ctx:
        singles = ctx.enter_context(tc.tile_pool(name="singles", bufs=1))
        vt_pool = ctx.enter_context(tc.tile_pool(name="vt_pool", bufs=NQC))
        exps_pool = ctx.enter_context(tc.tile_pool(name="exps_pool", bufs=6))
        sum_pool = ctx.enter_context(tc.tile_pool(name="sum_pool", bufs=2))
        out_pool = ctx.enter_context(tc.tile_pool(name="out_pool", bufs=2))
        ps_s = ctx.enter_context(tc.tile_pool(name="ps_s", bufs=2,
                                              space="PSUM"))
        ps_z = ctx.enter_context(tc.tile_pool(name="ps_z", bufs=1,
                                              space="PSUM"))
        ps_w = ctx.enter_context(tc.tile_pool(name="ps_w", bufs=2,
                                              space="PSUM"))

        # ---- x pieces first (512 cols lead), packed weights interleaved ----
        w_k = singles.tile([128, 2 * DK], f32r)    # [:,0:64]=kwT[0:128]
        w_q = singles.tile([128, 2 * DK], f32r)
        w_v = singles.tile([128, 2 * DV], f32r)

        k2 = singles.tile([128, HW], f32r)    # K/2 on both partition halves
        q2 = singles.tile([128, PQ], f32r)    # Q on both partition halves
        xf0 = singles.tile([128, HW], f32r)
        xf1 = singles.tile([128, HW], f32r)

        pieces = [(0, 512), (512, 1536), (1536, 2560), (2560, 3584),
                  (3584, 4096)]
        nc.sync.dma_start(
            out=w_k.rearrange("p (a k) -> p a k", a=2),
            in_=kwT.rearrange("(a p) k -> p a k", a=2, p=128))
        nc.scalar.dma_start(out=xf1[:, 0:512], in_=xb[128:256, 0:512])
        nc.sync.dma_start(out=xf0[:, 0:512], in_=xb[0:128, 0:512])
        nc.scalar.dma_start(
            out=w_v.rearrange("p (a v) -> p a v", a=2),
            in_=vwT.rearrange("(a p) v -> p a v", a=2, p=128))
        nc.sync.dma_start(
            out=w_q.rearrange("p (a k) -> p a k", a=2),
            in_=qwT.rearrange("(a p) k -> p a k", a=2, p=128))
        for lo, hi in pieces[1:]:
            nc.sync.dma_start(out=xf0[:, lo:hi], in_=xb[0:128, lo:hi])
            nc.scalar.dma_start(out=xf1[:, lo:hi], in_=xb[128:256, lo:hi])

        ones_f = singles.tile([128, 128], f32)
        nc.gpsimd.memset(ones_f, 1.0)
        ones_r = singles.tile([128, 128], f32r)
        nc.scalar.copy(ones_r, ones_f)
        ones_b = singles.tile([128, 128], bf16)
        nc.scalar.copy(ones_b, ones_f)

        vt = [None] * NQC

        def proj_chunk(g):
            """Projections for x cols [g*1024, (g+1)*1024)."""
            for j in range(2 * g, 2 * g + 2):     # 512-col groups
                sl = slice(j * PT, (j + 1) * PT)
                pk = ps_w.tile([64, PT], f32, name=f"pk{j}", tag="scr")
                nc.tensor.matmul(pk, w_k[:, 0:DK], xf0[:, sl],
                                 start=True, stop=False)
                nc.tensor.matmul(pk, w_k[:, DK:2 * DK], xf1[:, sl],
                                 start=False, stop=True)
                nc.vector.tensor_copy(k2[0:64, sl], pk)
                nc.scalar.copy(k2[64:128, sl], pk)
                if j < PQ // PT:
                    pq = ps_w.tile([64, PT], f32, name=f"pq{j}", tag="scr")
                    nc.tensor.matmul(pq, w_q[:, 0:DK], xf0[:, sl],
                                     start=True, stop=False)
                    nc.tensor.matmul(pq, w_q[:, DK:2 * DK], xf1[:, sl],
                                     start=False, stop=True)
                    nc.vector.tensor_copy(q2[0:64, sl], pq)
                    nc.scalar.copy(q2[64:128, sl], pq)
            for qc in range(8 * g, 8 * g + 8):
                sl = slice(qc * QC, (qc + 1) * QC)
                pv = ps_w.tile([QC, DV], f32, name=f"pv{qc}", tag="scr")
                nc.tensor.matmul(pv, xf0[:, sl], w_v[:, 0:DV],
                                 start=True, stop=False)
                nc.tensor.matmul(pv, xf1[:, sl], w_v[:, DV:2 * DV],
                                 start=False, stop=True)
                vt_t = vt_pool.tile([QC, DV], bf16, name=f"vt{qc}", tag="vt")
                if qc % 2 == 0:
                    nc.vector.tensor_copy(vt_t, pv)
                else:
                    nc.scalar.copy(vt_t, pv)
                vt[qc] = vt_t

        proj_chunk(0)

        # ---- attention main loop over query tiles, in chunk PAIRS ----
        EXP = mybir.ActivationFunctionType.Exp

        def s_pair(pt, i):
            qs = q2[:, pt * PT:(pt + 1) * PT]
            t = ps_s.tile([128, 2 * PT], f32, name=f"sp{pt}_{i}",
                          tag="spair")
            nc.tensor.matmul(t[:, 0:PT], k2[:, (2 * i) * QC:
                                             (2 * i + 1) * QC],
                             qs, start=True, stop=True)
            nc.tensor.matmul(t[:, PT:2 * PT], k2[:, (2 * i + 1) * QC:
                                                 (2 * i + 2) * QC],
                             qs, start=True, stop=True)
            return t

        def mk_exp(pt, i, t):
            e = exps_pool.tile([128, 2 * PT], bf16, name=f"e{pt}_{i}",
                               tag="exps")
            nc.scalar.activation(e, t, func=EXP)
            return e

        def preheat(pt):
            """Pair 0 of ptile pt via the scratch psum banks + pair 1's S,
            emitted while the previous ptile is still streaming, so the
            next ptile's Z matmuls can start with zero PE idle."""
            qs = q2[:, pt * PT:(pt + 1) * PT]
            sa = ps_w.tile([128, PT], f32, name=f"sa{pt}", tag="scr")
            nc.tensor.matmul(sa, k2[:, 0:QC], qs, start=True, stop=True)
            sb = ps_w.tile([128, PT], f32, name=f"sb{pt}", tag="scr")
            nc.tensor.matmul(sb, k2[:, QC:2 * QC], qs, start=True, stop=True)
            s1 = s_pair(pt, 1)
            e0 = exps_pool.tile([128, 2 * PT], bf16, name=f"e{pt}_0",
                                tag="exps")
            nc.scalar.activation(e0[:, 0:PT], sa, func=EXP)
            nc.scalar.activation(e0[:, PT:2 * PT], sb, func=EXP)
            return e0, s1

        tail_finish = None
        ph = preheat(0)
        for pt in range(NPT):
            pz0 = ps_z.tile([128, PT], f32, name=f"pz0_{pt}", tag="pz0")
            pz1 = ps_z.tile([128, PT], f32, name=f"pz1_{pt}", tag="pz1")
            acc_a = sum_pool.tile([128, 2 * PT], f32, name=f"acca{pt}",
                                  tag="acca")
            acc_b = sum_pool.tile([128, 2 * PT], f32, name=f"accb{pt}",
                                  tag="accb")
            acc_p = sum_pool.tile([128, 2 * PT], f32, name=f"accp{pt}",
                                  tag="accp")
            first = {"a": True, "b": True, "p": True}
            dve_parity = 0
            saved_e = {}

            pend = [ph[1]]
            E = {0: ph[0]}

            for i in range(PAIRS):
                if pt == 0 and i in (1, 5, 9):
                    proj_chunk(i // 4 + 1)
                if i + 2 < PAIRS:
                    pend.append(s_pair(pt, i + 2))
                if i + 1 < PAIRS:
                    E[i + 1] = mk_exp(pt, i + 1, pend.pop(0))
                if i == 0 and tail_finish is not None:
                    tail_finish[0]()  # evict prev pz before Z reuses banks
                if i == 2 and tail_finish is not None:
                    tail_finish[1]()  # prev denominator + normalize + store
                    tail_finish = None
                e = E.pop(i)
                c0, c1 = 2 * i, 2 * i + 1
                e0, e1 = e[:, 0:PT], e[:, PT:2 * PT]
                nc.tensor.matmul(pz0, vt[c0][:, 0:128], e0,
                                 start=(i == 0), stop=False)
                nc.tensor.matmul(pz0, vt[c1][:, 0:128], e1,
                                 start=False, stop=(i == PAIRS - 1))
                nc.tensor.matmul(pz1, vt[c0][:, 128:256], e0,
                                 start=(i == 0), stop=False)
                nc.tensor.matmul(pz1, vt[c1][:, 128:256], e1,
                                 start=False, stop=(i == PAIRS - 1))
                if i in PE_PAIRS:
                    saved_e[i] = e       # summed on PE after the last Z
                elif i in POOL_PAIRS:
                    if first["p"]:
                        nc.gpsimd.tensor_copy(acc_p, e)
                        first["p"] = False
                    else:
                        nc.gpsimd.tensor_add(acc_p, acc_p, e)
                else:
                    key = "a" if dve_parity == 0 else "b"
                    acc = acc_a if dve_parity == 0 else acc_b
                    dve_parity ^= 1
                    if first[key]:
                        nc.vector.tensor_copy(acc, e)
                        first[key] = False
                    else:
                        nc.vector.tensor_add(acc, acc, e)
                if i == 10:
                    # Pool folds its own accumulator while DVE still adds
                    acc_pr = sum_pool.tile([128, PT], f32, name=f"apr{pt}",
                                           tag="accpr")
                    nc.gpsimd.tensor_add(acc_pr, acc_p[:, 0:PT],
                                         acc_p[:, PT:2 * PT])
                if i == 13:
                    acc_t = sum_pool.tile([128, 2 * PT], f32, name=f"at{pt}",
                                          tag="acct")
                    nc.vector.tensor_add(acc_t, acc_a, acc_b)
                    acc_u = sum_pool.tile([128, PT], f32, name=f"au{pt}",
                                          tag="accu")
                    nc.vector.tensor_add(acc_u, acc_t[:, 0:PT],
                                         acc_t[:, PT:2 * PT])
                    accf = sum_pool.tile([128, PT], f32r, name=f"af{pt}",
                                         tag="accf")
                    nc.vector.tensor_add(accf, acc_u, acc_pr)
                if i == 14 and pt + 1 < NPT:
                    ph = preheat(pt + 1)


            # ---- deferred tail: evict pz, then denominator + normalize ----
            def make_tail(pt=pt, pz0=pz0, pz1=pz1, accf=accf,
                          saved_e=dict(saved_e)):
                st = {}

                def evict():
                    st["zr0"] = out_pool.tile([128, PT], f32,
                                              name=f"zr0_{pt}", tag="zr0")
                    st["zr1"] = out_pool.tile([128, PT], f32,
                                              name=f"zr1_{pt}", tag="zr1")
                    nc.vector.tensor_copy(st["zr0"], pz0)
                    nc.vector.tensor_copy(st["zr1"], pz1)

                def finish():
                    sums = ps_w.tile([128, PT], f32, name=f"sums{pt}",
                                     tag="scr")
                    nc.tensor.matmul(sums, ones_r, accf,
                                     start=True, stop=False)
                    for idx, j in enumerate(PE_PAIRS):
                        e = saved_e[j]
                        nc.tensor.matmul(sums, ones_b, e[:, 0:PT],
                                         start=False, stop=False)
                        nc.tensor.matmul(sums, ones_b, e[:, PT:2 * PT],
                                         start=False,
                                         stop=(idx == len(PE_PAIRS) - 1))
                    bcast = sum_pool.tile([128, PT], f32, name=f"bc{pt}",
                                          tag="bcast")
                    nc.vector.reciprocal_approx_fast(out=bcast, in_=sums)
                    out0 = out_pool.tile([128, PT], f32, name=f"o0_{pt}",
                                         tag="out0")
                    out1 = out_pool.tile([128, PT], f32, name=f"o1_{pt}",
                                         tag="out1")
                    nc.vector.tensor_mul(out0, st["zr0"], bcast)
                    nc.vector.tensor_mul(out1, st["zr1"], bcast)
                    # both on the sync ring: a DMA on the scalar ring blocks
                    # ACT's instruction queue behind the finish chain
                    nc.sync.dma_start(out=zout[0:128, pt * PT:(pt + 1) * PT],
                                      in_=out0)
                    nc.sync.dma_start(
                        out=zout[128:256, pt * PT:(pt + 1) * PT], in_=out1)
                return evict, finish

            if pt == NPT - 1:
                ev, fin = make_tail()
                ev()
                fin()
            else:
                tail_finish = make_tail()

    nc.compile()
    _cache["nc"] = nc
    return nc


def _to_f32r(a):
    """Round fp32 to fp32r (e8m11): RNE on the low 12 mantissa bits."""
    u = np.ascontiguousarray(a, np.float32).view(np.uint32)
    u = (u + np.uint32(0x7FF) + ((u >> np.uint32(12)) & np.uint32(1))) \
        & np.uint32(0xFFFFF000)
    return u.view(np.float32)


def _in_maps(x, q_w, k_w, v_w):
    xf = np.ascontiguousarray(x.reshape(B, DIN, HW), dtype=np.float32)
    qwT = _to_f32r(np.asarray(q_w, np.float32).T)
    # k_w halved: k2 holds K/2 on both partition halves, S contracts K=128
    kwT = _to_f32r(np.asarray(k_w, np.float32).T * 0.5)
    vwT = _to_f32r(np.asarray(v_w, np.float32).T)
    maps = []
    for c in range(N_CORES):
        b, half = divmod(c, 2)
        xbc = xf[b] if half == 0 else np.ascontiguousarray(
            np.roll(xf[b], -PQ, axis=1))
        maps.append({"xb": _to_f32r(xbc), "qwT": qwT, "kwT": kwT,
                     "vwT": vwT})
    return maps


def _gather(results):
    z = np.empty((B, DV, HW), np.float32)
    for c in range(N_CORES):
        b, half = divmod(c, 2)
        z[b][:, half * PQ:(half + 1) * PQ] = results[c]["zout"]
    return z.reshape(B, DV, H, W)


def _run(x, q_w, k_w, v_w, trace=False):
    from concourse import bass_utils
    nc = _build()
    res = bass_utils.run_bass_kernel_spmd(
        nc, _in_maps(x, q_w, k_w, v_w), core_ids=list(range(N_CORES)),
        trace=trace)
    return _gather(res.results), res


def kernel(x, q_w, k_w, v_w):
    z, _ = _run(x, q_w, k_w, v_w)
    return z


# revision 27
# speedup vs baseline: 1.0701x; 1.0701x over previous
"""Trainium2 Bass kernel for BasicAttention.

reference math (fp32):
  xf = x.reshape(b, din, hw)               # b=4, din=256, hw=4096
  Q = q_w @ xf   [b, 64, hw]
  K = k_w @ xf   [b, 64, hw]
  V = v_w @ xf   [b, 256, hw]
  S = Q^T K      [b, hw, hw]
  A = softmax(S, axis=-1)
  z = (A @ V^T)^T -> [b, 256, h, w]

Sharding: 8 cores = (batch b in 0..4) x (query half in 0..2). Each core gets
its batch's full xf with columns rotated so its 2048 queries come first
(attention is permutation-invariant over keys, so K/V built from the rotated
xf give identical outputs).

Per-core dataflow (fp32 PSUM, matmuls in float32r):
  - k2/q2 [128, *] hold K/2 and Q duplicated on both partition halves, so S
    matmuls contract over K=128: S = (K/2)^T Q + (K/2)^T Q.  K=64 fp32r
    matmuls would be silently row-grouped into a 2x-cost two-pass mode;
    K=128 streams at 1 row/cycle.  k_w is halved on the host.
  - S psums are [128, 1024] pairs (2 banks); ONE ACT exp per pair reads
    across both banks, halving ACT instruction count.
  - Z accumulates vt^T @ exp into two [128,512] psums; at ptile end they are
    evicted raw to SBUF (freeing the banks) and normalized there.
  - Softmax denominators: DVE accumulates exp pairs 0..13 on two interleaved
    chains (Pool takes 3 early pairs); pairs 14/15 are summed directly on PE
    with ones[128,128] matmuls accumulating into the same psum as the
    ones^T accf matmul, so the boundary chain after the last Z matmul is
    just 5 short matmuls -> reciprocal_approx_fast -> normalize muls.
"""

import sys
import os

sys.path.insert(0, "/opt/trn_rl_repo")

import numpy as np

B, DIN, H, W = 4, 256, 64, 64
HW = H * W            # 4096 keys
DK, DV = 64, 256
PQ = HW // 2          # 2048 queries per core
PT = 512              # query tile (psum free dim)
QC = 128              # key chunk (contraction tile)
NPT = PQ // PT        # 4
NQC = HW // QC        # 32
PAIRS = NQC // 2      # 16
POOL_PAIRS = (1, 3, 5, 7, 9)   # pair-adds done on GpSimd
PE_PAIRS = (12, 13, 14, 15)    # pairs summed directly on PE at ptile end
N_CORES = 8

_cache = {}


def _build():
    if "nc" in _cache:
        return _cache["nc"]

    from contextlib import ExitStack
    import concourse.tile as tile
    from concourse import bacc, mybir

    f32 = mybir.dt.float32
    f32r = mybir.dt.float32r
    bf16 = mybir.dt.bfloat16

    nc = bacc.Bacc("TRN2", target_bir_lowering=False, debug=False,
                   num_devices=N_CORES)

    xb = nc.dram_tensor("xb", [DIN, HW], f32r, kind="ExternalInput").ap()
    qwT = nc.dram_tensor("qwT", [DIN, DK], f32r, kind="ExternalInput").ap()
    kwT = nc.dram_tensor("kwT", [DIN, DK], f32r, kind="ExternalInput").ap()
    vwT = nc.dram_tensor("vwT", [DIN, DV], f32r, kind="ExternalInput").ap()
    zout = nc.dram_tensor("zout", [DV, PQ], f32, kind="ExternalOutput").ap()

    with tile.TileContext(nc) as tc, ExitStack() as ctx:
        singles = ctx.enter_context(tc.tile_pool(name="singles", bufs=1))
        vt_pool = ctx.enter_context(tc.tile_pool(name="vt_pool", bufs=NQC))
        exps_pool = ctx.enter_context(tc.tile_pool(name="exps_pool", bufs=8))
        sum_pool = ctx.enter_context(tc.tile_pool(name="sum_pool", bufs=2))
        out_pool = ctx.enter_context(tc.tile_pool(name="out_pool", bufs=2))
        ps_s = ctx.enter_context(tc.tile_pool(name="ps_s", bufs=2,
                                              space="PSUM"))
        ps_z = ctx.enter_context(tc.tile_pool(name="ps_z", bufs=1,
                                              space="PSUM"))
        ps_w = ctx.enter_context(tc.tile_pool(name="ps_w", bufs=2,
                                              space="PSUM"))

        # ---- x pieces first (512 cols lead), packed weights interleaved ----
        w_k = singles.tile([128, 2 * DK], f32r)    # [:,0:64]=kwT[0:128]
        w_q = singles.tile([128, 2 * DK], f32r)
        w_v = singles.tile([128, 2 * DV], f32r)

        k2 = singles.tile([128, HW], f32r)    # K/2 on both partition halves
        q2 = singles.tile([128, PQ], f32r)    # Q on both partition halves
        xf0 = singles.tile([128, HW], f32r)
        xf1 = singles.tile([128, HW], f32r)

        pieces = [(0, 512), (512, 1536), (1536, 2560), (2560, 3584),
                  (3584, 4096)]
        nc.sync.dma_start(
            out=w_k.rearrange("p (a k) -> p a k", a=2),
            in_=kwT.rearrange("(a p) k -> p a k", a=2, p=128))
        nc.scalar.dma_start(out=xf1[:, 0:512], in_=xb[128:256, 0:512])
        nc.sync.dma_start(out=xf0[:, 0:512], in_=xb[0:128, 0:512])
        nc.scalar.dma_start(
            out=w_v.rearrange("p (a v) -> p a v", a=2),
            in_=vwT.rearrange("(a p) v -> p a v", a=2, p=128))
        nc.sync.dma_start(
            out=w_q.rearrange("p (a k) -> p a k", a=2),
            in_=qwT.rearrange("(a p) k -> p a k", a=2, p=128))
        for lo, hi in pieces[1:]:
            nc.sync.dma_start(out=xf0[:, lo:hi], in_=xb[0:128, lo:hi])
            nc.scalar.dma_start(out=xf1[:, lo:hi], in_=xb[128:256, lo:hi])

        ones_f = singles.tile([128, 128], f32)
        nc.gpsimd.memset(ones_f, 1.0)
        ones_r = singles.tile([128, 128], f32r)
        nc.scalar.copy(ones_r, ones_f)
        ones_b = singles.tile([128, 128], bf16)
        nc.scalar.copy(ones_b, ones_f)

        vt = [None] * NQC

        def proj_chunk(g):
            """Projections for x cols [g*1024, (g+1)*1024)."""
            for j in range(2 * g, 2 * g + 2):     # 512-col groups
                sl = slice(j * PT, (j + 1) * PT)
                pk = ps_w.tile([64, PT], f32, name=f"pk{j}", tag="scr")
                nc.tensor.matmul(pk, w_k[:, 0:DK], xf0[:, sl],
                                 start=True, stop=False)
                nc.tensor.matmul(pk, w_k[:, DK:2 * DK], xf1[:, sl],
                                 start=False, stop=True)
                nc.vector.tensor_copy(k2[0:64, sl], pk)
                nc.scalar.copy(k2[64:128, sl], pk)
                if j < PQ // PT:
                    pq = ps_w.tile([64, PT], f32, name=f"pq{j}", tag="scr")
                    nc.tensor.matmul(pq, w_q[:, 0:DK], xf0[:, sl],
                                     start=True, stop=False)
                    nc.tensor.matmul(pq, w_q[:, DK:2 * DK], xf1[:, sl],
                                     start=False, stop=True)
                    nc.vector.tensor_copy(q2[0:64, sl], pq)
                    nc.scalar.copy(q2[64:128, sl], pq)
            for qc in range(8 * g, 8 * g + 8):
                sl = slice(qc * QC, (qc + 1) * QC)
                pv = ps_w.tile([QC, DV], f32, name=f"pv{qc}", tag="scr")
                nc.tensor.matmul(pv, xf0[:, sl], w_v[:, 0:DV],
                                 start=True, stop=False)
                nc.tensor.matmul(pv, xf1[:, sl], w_v[:, DV:2 * DV],
                                 start=False, stop=True)
                vt_t = vt_pool.tile([QC, DV], bf16, name=f"vt{qc}", tag="vt")
                if qc % 2 == 0:
                    nc.vector.tensor_copy(vt_t, pv)
                else:
                    nc.scalar.copy(vt_t, pv)
                vt[qc] = vt_t

        proj_chunk(0)

        # ---- attention main loop over query tiles, in chunk PAIRS ----
        EXP = mybir.ActivationFunctionType.Exp

        def s_pair(pt, i):
            qs = q2[:, pt * PT:(pt + 1) * PT]
            t = ps_s.tile([128, 2 * PT], f32, name=f"sp{pt}_{i}",
                          tag="spair")
            nc.tensor.matmul(t[:, 0:PT], k2[:, (2 * i) * QC:
                                             (2 * i + 1) * QC],
                             qs, start=True, stop=True)
            nc.tensor.matmul(t[:, PT:2 * PT], k2[:, (2 * i + 1) * QC:
                                                 (2 * i + 2) * QC],
                             qs, start=True, stop=True)
            return t

        def mk_exp(pt, i, t):
            e = exps_pool.tile([128, 2 * PT], bf16, name=f"e{pt}_{i}",
                               tag="exps")
            nc.scalar.activation(e, t, func=EXP)
            return e

        def preheat(pt):
            """Pair 0 of ptile pt via the scratch psum banks + pair 1's S,
            emitted while the previous ptile is still streaming, so the
            next ptile's Z matmuls can start with zero PE idle."""
            qs = q2[:, pt * PT:(pt + 1) * PT]
            sa = ps_w.tile([128, PT], f32, name=f"sa{pt}", tag="scr")
            nc.tensor.matmul(sa, k2[:, 0:QC], qs, start=True, stop=True)
            sb = ps_w.tile([128, PT], f32, name=f"sb{pt}", tag="scr")
            nc.tensor.matmul(sb, k2[:, QC:2 * QC], qs, start=True, stop=True)
            s1 = s_pair(pt, 1)
            e0 = exps_pool.tile([128, 2 * PT], bf16, name=f"e{pt}_0",
                                tag="exps")
            nc.scalar.activation(e0[:, 0:PT], sa, func=EXP)
            nc.scalar.activation(e0[:, PT:2 * PT], sb, func=EXP)
            return e0, s1

        def emit_sums(pt, accf, saved_e):
            sums = ps_w.tile([128, PT], f32, name=f"sums{pt}", tag="scr")
            nc.tensor.matmul(sums, ones_r, accf, start=True, stop=False)
            for idx, j in enumerate(PE_PAIRS):
                e = saved_e[j]
                nc.tensor.matmul(sums, ones_b, e[:, 0:PT],
                                 start=False, stop=False)
                nc.tensor.matmul(sums, ones_b, e[:, PT:2 * PT],
                                 start=False, stop=(idx == len(PE_PAIRS) - 1))
            return sums

        tail_finish = None
        last_sums = [None]
        ph = preheat(0)
        for pt in range(NPT):
            pz0 = ps_z.tile([128, PT], f32, name=f"pz0_{pt}", tag="pz0")
            pz1 = ps_z.tile([128, PT], f32, name=f"pz1_{pt}", tag="pz1")
            acc_a = sum_pool.tile([128, 2 * PT], f32, name=f"acca{pt}",
                                  tag="acca")
            acc_b = sum_pool.tile([128, 2 * PT], f32, name=f"accb{pt}",
                                  tag="accb")
            acc_p = sum_pool.tile([128, 2 * PT], f32, name=f"accp{pt}",
                                  tag="accp")
            first = {"a": True, "b": True, "p": True}
            dve_parity = 0
            saved_e = {}

            pend = [ph[1]]
            E = {0: ph[0]}

            for i in range(PAIRS):
                if pt == 0 and i in (1, 5, 9):
                    proj_chunk(i // 4 + 1)
                if i + 2 < PAIRS:
                    pend.append(s_pair(pt, i + 2))
                if i + 1 < PAIRS:
                    E[i + 1] = mk_exp(pt, i + 1, pend.pop(0))
                if i == 0 and tail_finish is not None:
                    tail_finish[0]()  # evict prev pz before Z reuses banks
                if i == 2 and tail_finish is not None:
                    tail_finish[1]()  # prev denominator + normalize + store
                    tail_finish = None
                e = E.pop(i)
                if pt == NPT - 1 and i == PAIRS - 1:
                    saved_e[i] = e
                    last_sums[0] = emit_sums(pt, accf, saved_e)
                c0, c1 = 2 * i, 2 * i + 1
                e0, e1 = e[:, 0:PT], e[:, PT:2 * PT]
                nc.tensor.matmul(pz0, vt[c0][:, 0:128], e0,
                                 start=(i == 0), stop=False)
                nc.tensor.matmul(pz0, vt[c1][:, 0:128], e1,
                                 start=False, stop=(i == PAIRS - 1))
                nc.tensor.matmul(pz1, vt[c0][:, 128:256], e0,
                                 start=(i == 0), stop=False)
                nc.tensor.matmul(pz1, vt[c1][:, 128:256], e1,
                                 start=False, stop=(i == PAIRS - 1))
                if i in PE_PAIRS:
                    saved_e[i] = e       # summed on PE after the last Z
                elif i in POOL_PAIRS:
                    if first["p"]:
                        nc.gpsimd.tensor_copy(acc_p, e)
                        first["p"] = False
                    else:
                        nc.gpsimd.tensor_add(acc_p, acc_p, e)
                else:
                    key = "a" if dve_parity == 0 else "b"
                    acc = acc_a if dve_parity == 0 else acc_b
                    dve_parity ^= 1
                    if first[key]:
                        nc.vector.tensor_copy(acc, e)
                        first[key] = False
                    else:
                        nc.vector.tensor_add(acc, acc, e)
                if i == 10:
                    # Pool folds its own accumulator while DVE still adds
                    acc_pr = sum_pool.tile([128, PT], f32, name=f"apr{pt}",
                                           tag="accpr")
                    nc.gpsimd.tensor_add(acc_pr, acc_p[:, 0:PT],
                                         acc_p[:, PT:2 * PT])
                if i == 11:
                    acc_t = sum_pool.tile([128, 2 * PT], f32, name=f"at{pt}",
                                          tag="acct")
                    nc.vector.tensor_add(acc_t, acc_a, acc_b)
                    acc_u = sum_pool.tile([128, PT], f32, name=f"au{pt}",
                                          tag="accu")
                    nc.vector.tensor_add(acc_u, acc_t[:, 0:PT],
                                         acc_t[:, PT:2 * PT])
                    accf = sum_pool.tile([128, PT], f32r, name=f"af{pt}",
                                         tag="accf")
                    nc.vector.tensor_add(accf, acc_u, acc_pr)
                if i == 14 and pt + 1 < NPT:
                    ph = preheat(pt + 1)



            # ---- deferred tail: evict pz, then denominator + normalize ----
            def make_tail(pt=pt, pz0=pz0, pz1=pz1, accf=accf,
                          saved_e=dict(saved_e)):
                st = {}

                def evict():
                    st["zr0"] = out_pool.tile([128, PT], f32,
                                              name=f"zr0_{pt}", tag="zr0")
                    st["zr1"] = out_pool.tile([128, PT], f32,
                                              name=f"zr1_{pt}", tag="zr1")
                    nc.vector.tensor_copy(st["zr0"], pz0)
                    nc.vector.tensor_copy(st["zr1"], pz1)

                def finish():
                    if last_sums[0] is not None:
                        sums = last_sums[0]
                    else:
                        sums = emit_sums(pt, accf, saved_e)
                    bcast = sum_pool.tile([128, PT], f32, name=f"bc{pt}",
                                          tag="bcast")
                    nc.vector.reciprocal_approx_fast(out=bcast, in_=sums)
                    out0 = out_pool.tile([128, PT], f32, name=f"o0_{pt}",
                                         tag="out0")
                    out1 = out_pool.tile([128, PT], f32, name=f"o1_{pt}",
                                         tag="out1")
                    nc.vector.tensor_mul(out0, st["zr0"], bcast)
                    nc.vector.tensor_mul(out1, st["zr1"], bcast)
                    # both on the sync ring: a DMA on the scalar ring blocks
                    # ACT's instruction queue behind the finish chain
                    nc.sync.dma_start(out=zout[0:128, pt * PT:(pt + 1) * PT],
                                      in_=out0)
                    nc.sync.dma_start(
                        out=zout[128:256, pt * PT:(pt + 1) * PT], in_=out1)
                return evict, finish

            if pt == NPT - 1:
                ev, fin = make_tail()
                ev()
                fin()
            else:
                tail_finish = make_tail()

    nc.compile()
    _cache["nc"] = nc
    return nc


def _to_f32r(a):
    """Round fp32 to fp32r (e8m11): RNE on the low 12 mantissa bits."""
    u = np.ascontiguousarray(a, np.float32).view(np.uint32)
    u = (u + np.uint32(0x7FF) + ((u >> np.uint32(12)) & np.uint32(1))) \
        & np.uint32(0xFFFFF000)
    return u.view(np.float32)


def _in_maps(x, q_w, k_w, v_w):
    xf = np.ascontiguousarray(x.reshape(B, DIN, HW), dtype=np.float32)
    qwT = _to_f32r(np.asarray(q_w, np.float32).T)
    # k_w halved: k2 holds K/2 on both partition halves, S contracts K=128
    kwT = _to_f32r(np.asarray(k_w, np.float32).T * 0.5)
    vwT = _to_f32r(np.asarray(v_w, np.float32).T)
    maps = []
    for c in range(N_CORES):
        b, half = divmod(c, 2)
        xbc = xf[b] if half == 0 else np.ascontiguousarray(
            np.roll(xf[b], -PQ, axis=1))
        maps.append({"xb": _to_f32r(xbc), "qwT": qwT, "kwT": kwT,
                     "vwT": vwT})
    return maps


def _gather(results):
    z = np.empty((B, DV, HW), np.float32)
    for c in range(N_CORES):
        b, half = divmod(c, 2)
        z[b][:, half * PQ:(half + 1) * PQ] = results[c]["zout"]
    return z.reshape(B, DV, H, W)


def _run(x, q_w, k_w, v_w, trace=False):
    from concourse import bass_utils
    nc = _build()
    res = bass_utils.run_bass_kernel_spmd(
        nc, _in_maps(x, q_w, k_w, v_w), core_ids=list(range(N_CORES)),
        trace=trace)
    return _gather(res.results), res


def kernel(x, q_w, k_w, v_w):
    z, _ = _run(x, q_w, k_w, v_w)
    return z


# revision 28
# speedup vs baseline: 1.2453x; 1.1637x over previous
"""Trainium2 Bass kernel for BasicAttention.

reference math (fp32):
  xf = x.reshape(b, din, hw)               # b=4, din=256, hw=4096
  Q = q_w @ xf   [b, 64, hw]
  K = k_w @ xf   [b, 64, hw]
  V = v_w @ xf   [b, 256, hw]
  S = Q^T K      [b, hw, hw]
  A = softmax(S, axis=-1)
  z = (A @ V^T)^T -> [b, 256, h, w]

Sharding: 8 cores = (batch b in 0..4) x (query half in 0..2). Each core gets
its batch's full xf with columns rotated so its 2048 queries come first
(attention is permutation-invariant over keys, so K/V built from the rotated
xf give identical outputs).

Per-core dataflow (fp32 PSUM, matmuls in float32r):
  - k2/q2 [128, *] hold K/2 and Q duplicated on both partition halves, so S
    matmuls contract over K=128: S = (K/2)^T Q + (K/2)^T Q.  K=64 fp32r
    matmuls would be silently row-grouped into a 2x-cost two-pass mode;
    K=128 streams at 1 row/cycle.  k_w is halved on the host.
  - S psums are [128, 1024] pairs (2 banks); ONE ACT exp per pair reads
    across both banks, halving ACT instruction count.
  - Z accumulates vt^T @ exp into two [128,512] psums; at ptile end they are
    evicted raw to SBUF (freeing the banks) and normalized there.
  - Softmax denominators: DVE accumulates exp pairs 0..13 on two interleaved
    chains (Pool takes 3 early pairs); pairs 14/15 are summed directly on PE
    with ones[128,128] matmuls accumulating into the same psum as the
    ones^T accf matmul, so the boundary chain after the last Z matmul is
    just 5 short matmuls -> reciprocal_approx_fast -> normalize muls.
"""

import sys
import os

sys.path.insert(0, "/opt/trn_rl_repo")

import numpy as np

B, DIN, H, W = 4, 256, 64, 64
HW = H * W            # 4096 keys
DK, DV = 64, 256
PQ = HW // 2          # 2048 queries per core
PT = 512              # query tile (psum free dim)
QC = 128              # key chunk (contraction tile)
NPT = PQ // PT        # 4
NQC = HW // QC        # 32
PAIRS = NQC // 2      # 16
POOL_PAIRS = (1, 3, 5, 7, 9)   # pair-adds done on GpSimd
PE_PAIRS = (12, 13, 14, 15)    # pairs summed directly on PE at ptile end
N_CORES = 8

_cache = {}


def _build():
    if "nc" in _cache:
        return _cache["nc"]

    from contextlib import ExitStack
    import concourse.tile as tile
    from concourse import bacc, mybir

    f32 = mybir.dt.float32
    f32r = mybir.dt.float32r
    bf16 = mybir.dt.bfloat16

    nc = bacc.Bacc("TRN2", target_bir_lowering=False, debug=False,
                   num_devices=N_CORES)

    xb = nc.dram_tensor("xb", [DIN, HW], f32r, kind="ExternalInput").ap()
    qwT = nc.dram_tensor("qwT", [DIN, DK], f32r, kind="ExternalInput").ap()
    kwT = nc.dram_tensor("kwT", [DIN, DK], f32r, kind="ExternalInput").ap()
    vwT = nc.dram_tensor("vwT", [DIN, DV], f32r, kind="ExternalInput").ap()
    zout = nc.dram_tensor("zout", [DV, PQ], f32, kind="ExternalOutput").ap()

    with tile.TileContext(nc) as tc, ExitStack() as ctx:
        singles = ctx.enter_context(tc.tile_pool(name="singles", bufs=1))
        vt_pool = ctx.enter_context(tc.tile_pool(name="vt_pool", bufs=NQC))
        exps_pool = ctx.enter_context(tc.tile_pool(name="exps_pool", bufs=8))
        sum_pool = ctx.enter_context(tc.tile_pool(name="sum_pool", bufs=2))
        out_pool = ctx.enter_context(tc.tile_pool(name="out_pool", bufs=2))
        ps_s = ctx.enter_context(tc.tile_pool(name="ps_s", bufs=2,
                                              space="PSUM"))
        ps_z = ctx.enter_context(tc.tile_pool(name="ps_z", bufs=1,
                                              space="PSUM"))
        ps_w = ctx.enter_context(tc.tile_pool(name="ps_w", bufs=2,
                                              space="PSUM"))

        # ---- x pieces first (512 cols lead), packed weights interleaved ----
        w_k = singles.tile([128, 2 * DK], f32r)    # [:,0:64]=kwT[0:128]
        w_q = singles.tile([128, 2 * DK], f32r)
        w_v = singles.tile([128, 2 * DV], f32r)

        k2 = singles.tile([128, HW], f32r)    # K/2 on both partition halves
        q2 = singles.tile([128, PQ], f32r)    # Q on both partition halves
        xf0 = singles.tile([128, HW], f32r)
        xf1 = singles.tile([128, HW], f32r)

        pieces = [(0, 512), (512, 1536), (1536, 2560), (2560, 3584),
                  (3584, 4096)]
        nc.sync.dma_start(
            out=w_k.rearrange("p (a k) -> p a k", a=2),
            in_=kwT.rearrange("(a p) k -> p a k", a=2, p=128))
        nc.scalar.dma_start(out=xf1[:, 0:512], in_=xb[128:256, 0:512])
        nc.sync.dma_start(out=xf0[:, 0:512], in_=xb[0:128, 0:512])
        nc.scalar.dma_start(
            out=w_v.rearrange("p (a v) -> p a v", a=2),
            in_=vwT.rearrange("(a p) v -> p a v", a=2, p=128))
        nc.sync.dma_start(
            out=w_q.rearrange("p (a k) -> p a k", a=2),
            in_=qwT.rearrange("(a p) k -> p a k", a=2, p=128))
        for lo, hi in pieces[1:]:
            nc.sync.dma_start(out=xf0[:, lo:hi], in_=xb[0:128, lo:hi])
            nc.scalar.dma_start(out=xf1[:, lo:hi], in_=xb[128:256, lo:hi])

        ones_f = singles.tile([128, 128], f32)
        nc.gpsimd.memset(ones_f, 1.0)
        ones_r = singles.tile([128, 128], f32r)
        nc.scalar.copy(ones_r, ones_f)
        ones_b = singles.tile([128, 128], bf16)
        nc.scalar.copy(ones_b, ones_f)

        vt = [None] * NQC

        def proj_chunk(g):
            """Projections for x cols [g*1024, (g+1)*1024)."""
            for j in range(2 * g, 2 * g + 2):     # 512-col groups
                sl = slice(j * PT, (j + 1) * PT)
                pk = ps_w.tile([64, PT], f32, name=f"pk{j}", tag="scr")
                nc.tensor.matmul(pk, w_k[:, 0:DK], xf0[:, sl],
                                 start=True, stop=False)
                nc.tensor.matmul(pk, w_k[:, DK:2 * DK], xf1[:, sl],
                                 start=False, stop=True)
                nc.vector.tensor_copy(k2[0:64, sl], pk)
                nc.scalar.copy(k2[64:128, sl], pk)
                if j < PQ // PT:
                    pq = ps_w.tile([64, PT], f32, name=f"pq{j}", tag="scr")
                    nc.tensor.matmul(pq, w_q[:, 0:DK], xf0[:, sl],
                                     start=True, stop=False)
                    nc.tensor.matmul(pq, w_q[:, DK:2 * DK], xf1[:, sl],
                                     start=False, stop=True)
                    nc.vector.tensor_copy(q2[0:64, sl], pq)
                    nc.scalar.copy(q2[64:128, sl], pq)
            for qc in range(8 * g, 8 * g + 8):
                sl = slice(qc * QC, (qc + 1) * QC)
                pv = ps_w.tile([QC, DV], f32, name=f"pv{qc}", tag="scr")
                nc.tensor.matmul(pv, xf0[:, sl], w_v[:, 0:DV],
                                 start=True, stop=False)
                nc.tensor.matmul(pv, xf1[:, sl], w_v[:, DV:2 * DV],
                                 start=False, stop=True)
                vt_t = vt_pool.tile([QC, DV], bf16, name=f"vt{qc}", tag="vt")
                if qc % 2 == 0:
                    nc.vector.tensor_copy(vt_t, pv)
                else:
                    nc.scalar.copy(vt_t, pv)
                vt[qc] = vt_t

        proj_chunk(0)

        # ---- attention main loop over query tiles, in chunk PAIRS ----
        EXP = mybir.ActivationFunctionType.Exp

        def s_pair(pt, i):
            qs = q2[:, pt * PT:(pt + 1) * PT]
            t = ps_s.tile([128, 2 * PT], f32, name=f"sp{pt}_{i}",
                          tag="spair")
            nc.tensor.matmul(t[:, 0:PT], k2[:, (2 * i) * QC:
                                             (2 * i + 1) * QC],
                             qs, start=True, stop=True)
            nc.tensor.matmul(t[:, PT:2 * PT], k2[:, (2 * i + 1) * QC:
                                                 (2 * i + 2) * QC],
                             qs, start=True, stop=True)
            return t

        def mk_exp(pt, i, t):
            e = exps_pool.tile([128, 2 * PT], bf16, name=f"e{pt}_{i}",
                               tag="exps")
            nc.scalar.activation(e, t, func=EXP)
            return e

        def preheat(pt):
            """Pair 0 of ptile pt via the scratch psum banks + pair 1's S,
            emitted while the previous ptile is still streaming, so the
            next ptile's Z matmuls can start with zero PE idle."""
            qs = q2[:, pt * PT:(pt + 1) * PT]
            sa = ps_w.tile([128, PT], f32, name=f"sa{pt}", tag="scr")
            nc.tensor.matmul(sa, k2[:, 0:QC], qs, start=True, stop=True)
            sb = ps_w.tile([128, PT], f32, name=f"sb{pt}", tag="scr")
            nc.tensor.matmul(sb, k2[:, QC:2 * QC], qs, start=True, stop=True)
            s1 = s_pair(pt, 1)
            e0 = exps_pool.tile([128, 2 * PT], bf16, name=f"e{pt}_0",
                                tag="exps")
            nc.scalar.activation(e0[:, 0:PT], sa, func=EXP)
            nc.scalar.activation(e0[:, PT:2 * PT], sb, func=EXP)
            return e0, s1

        def emit_sums(pt, accf, saved_e):
            sums = ps_w.tile([128, PT], f32, name=f"sums{pt}", tag="scr")
            nc.tensor.matmul(sums, ones_r, accf, start=True, stop=False)
            for idx, j in enumerate(PE_PAIRS):
                e = saved_e[j]
                nc.tensor.matmul(sums, ones_b, e[:, 0:PT],
                                 start=False, stop=False)
                nc.tensor.matmul(sums, ones_b, e[:, PT:2 * PT],
                                 start=False, stop=(idx == len(PE_PAIRS) - 1))
            return sums

        tail_finish = None
        last_sums = [None]
        ph = preheat(0)
        for pt in range(NPT):
            pz0 = ps_z.tile([128, PT], f32, name=f"pz0_{pt}", tag="pz0")
            pz1 = ps_z.tile([128, PT], f32, name=f"pz1_{pt}", tag="pz1")
            acc_a = sum_pool.tile([128, 2 * PT], f32, name=f"acca{pt}",
                                  tag="acca")
            acc_b = sum_pool.tile([128, 2 * PT], f32, name=f"accb{pt}",
                                  tag="accb")
            acc_p = sum_pool.tile([128, 2 * PT], f32, name=f"accp{pt}",
                                  tag="accp")
            first = {"a": True, "b": True, "p": True}
            dve_parity = 0
            saved_e = {}

            pend = [ph[1]]
            E = {0: ph[0]}

            for i in range(PAIRS):
                if pt == 0 and i in (1, 5, 9):
                    proj_chunk(i // 4 + 1)
                if i + 2 < PAIRS:
                    pend.append(s_pair(pt, i + 2))
                if i + 1 < PAIRS:
                    E[i + 1] = mk_exp(pt, i + 1, pend.pop(0))
                if i == 0 and tail_finish is not None:
                    tail_finish[0]()  # evict prev pz before Z reuses banks
                if i == 2 and tail_finish is not None:
                    tail_finish[1]()  # prev denominator + normalize + store
                    tail_finish = None
                e = E.pop(i)
                if pt == NPT - 1 and i == PAIRS - 1:
                    saved_e[i] = e
                    last_sums[0] = emit_sums(pt, accf, saved_e)
                c0, c1 = 2 * i, 2 * i + 1
                e0, e1 = e[:, 0:PT], e[:, PT:2 * PT]
                nc.tensor.matmul(pz0, vt[c0][:, 0:128], e0,
                                 start=(i == 0), stop=False)
                nc.tensor.matmul(pz0, vt[c1][:, 0:128], e1,
                                 start=False, stop=(i == PAIRS - 1))
                nc.tensor.matmul(pz1, vt[c0][:, 128:256], e0,
                                 start=(i == 0), stop=False)
                nc.tensor.matmul(pz1, vt[c1][:, 128:256], e1,
                                 start=False, stop=(i == PAIRS - 1))
                if i in PE_PAIRS:
                    saved_e[i] = e       # summed on PE after the last Z
                elif i in POOL_PAIRS:
                    if first["p"]:
                        nc.gpsimd.tensor_copy(acc_p, e)
                        first["p"] = False
                    else:
                        nc.gpsimd.tensor_add(acc_p, acc_p, e)
                else:
                    key = "a" if dve_parity == 0 else "b"
                    acc = acc_a if dve_parity == 0 else acc_b
                    dve_parity ^= 1
                    if first[key]:
                        nc.vector.tensor_copy(acc, e)
                        first[key] = False
                    else:
                        nc.vector.tensor_add(acc, acc, e)
                if i == 10:
                    # Pool folds its own accumulator while DVE still adds
                    acc_pr = sum_pool.tile([128, PT], f32, name=f"apr{pt}",
                                           tag="accpr")
                    nc.gpsimd.tensor_add(acc_pr, acc_p[:, 0:PT],
                                         acc_p[:, PT:2 * PT])
                if i == 11:
                    acc_t = sum_pool.tile([128, 2 * PT], f32, name=f"at{pt}",
                                          tag="acct")
                    nc.vector.tensor_add(acc_t, acc_a, acc_b)
                    acc_u = sum_pool.tile([128, PT], f32, name=f"au{pt}",
                                          tag="accu")
                    nc.vector.tensor_add(acc_u, acc_t[:, 0:PT],
                                         acc_t[:, PT:2 * PT])
                    accf = sum_pool.tile([128, PT], f32r, name=f"af{pt}",
                                         tag="accf")
                    nc.vector.tensor_add(accf, acc_u, acc_pr)
                if i == 14 and pt + 1 < NPT:
                    ph = preheat(pt + 1)



            # ---- deferred tail: evict pz, then denominator + normalize ----
            def make_tail(pt=pt, pz0=pz0, pz1=pz1, accf=accf,
                          saved_e=dict(saved_e)):
                st = {}
                last = pt == NPT - 1

                def evict():
                    st["zr0"] = out_pool.tile([128, PT], f32,
                                              name=f"zr0_{pt}", tag="zr0")
                    st["zr1"] = out_pool.tile([128, PT], f32,
                                              name=f"zr1_{pt}", tag="zr1")
                    nc.vector.tensor_copy(st["zr0"], pz0)
                    if last:
                        nc.scalar.copy(st["zr1"], pz1)
                    else:
                        nc.vector.tensor_copy(st["zr1"], pz1)

                def finish():
                    if last_sums[0] is not None:
                        sums = last_sums[0]
                    else:
                        sums = emit_sums(pt, accf, saved_e)
                    bcast = sum_pool.tile([128, PT], f32, name=f"bc{pt}",
                                          tag="bcast")
                    nc.vector.reciprocal_approx_fast(out=bcast, in_=sums)
                    out0 = out_pool.tile([128, PT], f32, name=f"o0_{pt}",
                                         tag="out0")
                    out1 = out_pool.tile([128, PT], f32, name=f"o1_{pt}",
                                         tag="out1")
                    nc.vector.tensor_mul(out0, st["zr0"], bcast)
                    nc.vector.tensor_mul(out1, st["zr1"], bcast)
                    # both on the sync ring: a DMA on the scalar ring blocks
                    # ACT's instruction queue behind the finish chain
                    nc.sync.dma_start(out=zout[0:128, pt * PT:(pt + 1) * PT],
                                      in_=out0)
                    eng = nc.scalar if last else nc.sync
                    eng.dma_start(
                        out=zout[128:256, pt * PT:(pt + 1) * PT], in_=out1)
                return evict, finish

            if pt == NPT - 1:
                ev, fin = make_tail()
                ev()
                fin()
            else:
                tail_finish = make_tail()

    nc.compile()
    _cache["nc"] = nc
    return nc


def _to_f32r(a):
    """Round fp32 to fp32r (e8m11): RNE on the low 12 mantissa bits."""
    u = np.ascontiguousarray(a, np.float32).view(np.uint32)
    u = (u + np.uint32(0x7FF) + ((u >> np.uint32(12)) & np.uint32(1))) \
        & np.uint32(0xFFFFF000)
    return u.view(np.float32)


def _in_maps(x, q_w, k_w, v_w):
    xf = np.ascontiguousarray(x.reshape(B, DIN, HW), dtype=np.float32)
    qwT = _to_f32r(np.asarray(q_w, np.float32).T)
    # k_w halved: k2 holds K/2 on both partition halves, S contracts K=128
    kwT = _to_f32r(np.asarray(k_w, np.float32).T * 0.5)
    vwT = _to_f32r(np.asarray(v_w, np.float32).T)
    maps = []
    for c in range(N_CORES):
        b, half = divmod(c, 2)
        xbc = xf[b] if half == 0 else np.ascontiguousarray(
            np.roll(xf[b], -PQ, axis=1))
        maps.append({"xb": _to_f32r(xbc), "qwT": qwT, "kwT": kwT,
                     "vwT": vwT})
    return maps


def _gather(results):
    z = np.empty((B, DV, HW), np.float32)
    for c in range(N_CORES):
        b, half = divmod(c, 2)
        z[b][:, half * PQ:(half + 1) * PQ] = results[c]["zout"]
    return z.reshape(B, DV, H, W)


def _run(x, q_w, k_w, v_w, trace=False):
    from concourse import bass_utils
    nc = _build()
    res = bass_utils.run_bass_kernel_spmd(
        nc, _in_maps(x, q_w, k_w, v_w), core_ids=list(range(N_CORES)),
        trace=trace)
    return _gather(res.results), res


def kernel(x, q_w, k_w, v_w):
    z, _ = _run(x, q_w, k_w, v_w)
    return z
